# revision 1
# baseline (speedup 1.0000x reference)
"""3-layer GAT (GATConv x3 + log_softmax) on 8 Trainium2 NeuronCores.

Strategy (dst-sharded, edge-parallel within core):
- Host: nodes are permuted (per-core contiguous dst ranges, degree-sorted
  within each core so 128-node windows have uniform max degree). Edges are
  grouped by (core, window, partition=dst%128, k=slot).
- Device, per layer l: build a full node table [N_pad, 136] f32 where row n =
  [payload(128) | a_src_l (<=4) | a_dst_l (<=4) | pad]; payload is h1 for
  layer 1 and elu(out_{l-1}) for layers 2/3 (W_l applied POST-aggregation,
  which commutes with the weighted sum). Each core computes its own 6272 rows
  and an AllGather replicates the table.
- Edge phase per 128-dst-node window: indirect-DMA gather of src rows (one
  128-row call per k-slot; OOB index = padding, skipped by bounds check),
  scores/softmax/weighted-sum on vector+scalar engines in dst-major layout
  (partition = dst node). No segment max needed: scores are O(+-5) in f32.
- log_softmax on the final 9 logits; host inverse-permutes rows.
"""
import numpy as np

import concourse.bass as bass
import concourse.mybir as mybir
import concourse.tile as tile
from concourse.masks import make_identity

# ---- problem constants (hardcoded per contest rules) ----
N = 50000
E = 800000
F_IN = 300
HEADS = 4
PER_HEAD = 32
HID = 128
N_CLASSES = 9
NEG_SLOPE = 0.2

NC_ = 8
NPER = 6272          # nodes per core (49 * 128)
NPAD = NC_ * NPER    # 50176
P = 128
NWIN = NPER // P     # 49
KC = 16              # k-slots processed per chunk
DT = mybir.dt.float32
OOB = 1 << 20

f32 = mybir.dt.float32
AF = mybir.ActivationFunctionType


# ----------------------------------------------------------------------------
# host-side graph prep
# ----------------------------------------------------------------------------
def prep_graph(edge_index):
    s = np.asarray(edge_index[0], dtype=np.int64)
    d = np.asarray(edge_index[1], dtype=np.int64)
    deg = np.bincount(d, minlength=N)

    # permutation: per-core contiguous ranges, degree-desc within core
    old_of_new = np.full(NPAD, -1, dtype=np.int64)
    new_of_old = np.full(N, -1, dtype=np.int64)
    bounds = [min(c * NPER, N) for c in range(NC_ + 1)]
    for c in range(NC_):
        lo, hi = bounds[c], bounds[c + 1]
        nodes = np.arange(lo, hi)
        order = nodes[np.argsort(-deg[nodes], kind="stable")]
        old_of_new[c * NPER : c * NPER + len(order)] = order
        new_of_old[order] = c * NPER + np.arange(len(order))

    s_new = new_of_old[s]
    d_new = new_of_old[d]

    cores = []
    order_all = np.lexsort((s_new, d_new))  # sort edges by (dst_new, src_new)
    s_sorted = s_new[order_all]
    d_sorted = d_new[order_all]
    # edge ranges per dst node (csr)
    starts = np.searchsorted(d_sorted, np.arange(NPAD))
    ends = np.searchsorted(d_sorted, np.arange(NPAD) + 1)

    for c in range(NC_):
        Ks = []
        idx_cols = []
        msk_cols = []
        for w in range(NWIN):
            base = c * NPER + w * P
            degs = ends[base : base + P] - starts[base : base + P]
            K = int(degs.max()) if len(degs) else 0
            Ks.append(K)
            if K == 0:
                continue
            iw = np.full((P, K), OOB, dtype=np.int32)
            mw = np.zeros((P, K), dtype=np.float32)
            for p in range(P):
                a, b = starts[base + p], ends[base + p]
                iw[p, : b - a] = s_sorted[a:b]
                mw[p, : b - a] = 1.0
            idx_cols.append(iw)
            msk_cols.append(mw)
        idx = (
            np.concatenate(idx_cols, axis=1)
            if idx_cols
            else np.zeros((P, 1), np.int32)
        )
        msk = (
            np.concatenate(msk_cols, axis=1)
            if msk_cols
            else np.zeros((P, 1), np.float32)
        )
        cores.append({"K": Ks, "idx": idx, "mask": msk})
    return cores, old_of_new, new_of_old


# ----------------------------------------------------------------------------
# device kernel builder
# ----------------------------------------------------------------------------
def split_excess_waits(nc, max_waits=1):
    """This walrus build allows at most one sync-wait per instruction in the
    lowerings we hit; move excess (and any wait on an indirect DMA) onto
    inserted same-engine NoOps — equivalent on an in-order sequencer."""
    import copy

    n = 0
    for f in nc.m.functions:
        for blk in f.blocks:
            new_insts = []
            for ins in blk.instructions:
                need = (
                    ins.sync_info is not None and len(ins.sync_info.on_wait) > max_waits
                ) or (
                    isinstance(ins, mybir.InstDMACopy)
                    and getattr(ins, "queue", "") == "qPoolDynamic"
                    and ins.sync_info is not None
                    and len(ins.sync_info.on_wait) > 0
                )
                if need:
                    for w in list(ins.sync_info.on_wait):
                        noop = mybir.InstNoOp(
                            name=f"wait_split_{n}",
                            text_hint="wait_split",
                            bass_nofuse=True,
                        )
                        n += 1
                        noop.engine = ins.engine
                        si = copy.deepcopy(ins.sync_info)
                        si.on_update = type(si.on_update)()
                        si.on_wait = type(si.on_wait)([copy.deepcopy(w)])
                        noop.sync_info = si
                        new_insts.append(noop)
                    ins.sync_info.on_wait = type(ins.sync_info.on_wait)()
                new_insts.append(ins)
            if n:
                blk.instructions = new_insts
    return n


def build_nc(Ks, S):
    """One SPMD program; per-core data differs only in tensor contents.
    Ks: per-window K (max over cores so shapes are uniform), len NWIN.
    S: total k-columns = sum(Ks)."""
    nc = bass.Bass()
    TW = 136  # table row width

    xT = nc.declare_dram_parameter("xT", [F_IN, NPER], f32, isOutput=False)
    w1aug = nc.declare_dram_parameter("w1aug", [F_IN, TW], f32, isOutput=False)
    w2aug = nc.declare_dram_parameter("w2aug", [HID, TW], f32, isOutput=False)
    w3 = nc.declare_dram_parameter("w3", [HID, N_CLASSES], f32, isOutput=False)
    w3a = nc.declare_dram_parameter("w3a", [HID, 8], f32, isOutput=False)
    bias1 = nc.declare_dram_parameter("bias1", [P, HID], f32, isOutput=False)
    bias2 = nc.declare_dram_parameter("bias2", [P, HID], f32, isOutput=False)
    bias3 = nc.declare_dram_parameter("bias3", [P, N_CLASSES], f32, isOutput=False)
    idx_in = nc.declare_dram_parameter("idx", [P, S], mybir.dt.int32, isOutput=False)
    mask_in = nc.declare_dram_parameter("mask", [P, S], f32, isOutput=False)
    out_ext = nc.declare_dram_parameter("out", [NPER, N_CLASSES], f32, isOutput=True)

    shard = [nc.dram_tensor(f"shard{l}", [NPER, TW], f32) for l in range(3)]
    table_sh = [
        nc.dram_tensor(f"table_sh{l}", [NPAD, TW], f32, addr_space="Shared")
        for l in range(3)
    ]
    table = [nc.dram_tensor(f"table{l}", [NPAD, TW], f32) for l in range(3)]

    rg = [list(range(NC_))]

    with tile.TileContext(nc) as tc:
        with (
            tc.tile_pool(name="resident", bufs=1) as rp,
            tc.tile_pool(name="sbuf", bufs=3) as pool,
            tc.tile_pool(name="gp", bufs=4) as gpool,
            tc.tile_pool(name="mp", bufs=2) as mpool,
            tc.tile_pool(name="psum", bufs=2, space="PSUM") as pp,
            tc.tile_pool(name="psum_t", bufs=2, space="PSUM") as ppt,
        ):
            # ---------- residents ----------
            idx_sb = rp.tile([P, S], mybir.dt.int32)
            nc.gpsimd.dma_start(out=idx_sb[:], in_=idx_in[:])
            mask_sb = rp.tile([P, S], f32)
            nc.sync.dma_start(out=mask_sb[:], in_=mask_in[:])
            ident = rp.tile([P, P], f32)
            make_identity(nc, ident[:])
            b1_sb = rp.tile([P, HID], f32)
            nc.sync.dma_start(out=b1_sb[:], in_=bias1[:])
            b2_sb = rp.tile([P, HID], f32)
            nc.sync.dma_start(out=b2_sb[:], in_=bias2[:])
            b3_sb = rp.tile([P, N_CLASSES], f32)
            nc.sync.dma_start(out=b3_sb[:], in_=bias3[:])
            w2aug_sb = rp.tile([P, TW], f32)
            nc.sync.dma_start(out=w2aug_sb[:], in_=w2aug[:])
            w3_sb = rp.tile([P, N_CLASSES], f32)
            nc.sync.dma_start(out=w3_sb[:], in_=w3[:])
            w3a_sb = rp.tile([P, 8], f32)
            nc.sync.dma_start(out=w3a_sb[:], in_=w3a[:])
            w1_sb = rp.tile([P, 3 * TW], f32)  # 3 k-chunks of w1aug
            for kc in range(3):
                kd = min(P, F_IN - kc * P)
                nc.sync.dma_start(
                    out=w1_sb[:kd, kc * TW : kc * TW + TW],
                    in_=w1aug[kc * P : kc * P + kd, :],
                )
            # per-layer a_dst of own nodes [P, NWIN*4]; layer1 filled in node
            # transform, layers 2/3 during previous edge phase
            adst = [rp.tile([P, NWIN * 4], f32, name=f"adst{l}") for l in range(3)]

            bound = nc.gpsimd.to_reg(NPAD - 1)

            # memset gather pool slots once (avoid NaN garbage in pad slots)
            gz = [gpool.tile([P, KC, TW], f32, tag="g", name=f"gz{i}") for i in range(4)]
            for t in gz:
                nc.gpsimd.memset(t[:], 0.0)

            # ---------- layer-1 node transform ----------
            # shard0 rows = [h1 | a_src1 | a_dst1], h1 = x @ W1 etc.
            for t in range(NWIN):
                hpsum = pp.tile([P, TW], f32, tag="hpsum")
                for kc in range(3):
                    kd = min(P, F_IN - kc * P)
                    xt = pool.tile([P, P], f32, tag="xt")
                    nc.sync.dma_start(
                        out=xt[:kd, :],
                        in_=xT[kc * P : kc * P + kd, t * P : (t + 1) * P],
                    )
                    nc.tensor.matmul(
                        out=hpsum[:],
                        lhsT=xt[:kd, :],
                        rhs=w1_sb[:kd, kc * TW : kc * TW + TW],
                        start=(kc == 0),
                        stop=(kc == 2),
                    )
                hrow = pool.tile([P, TW], f32, tag="hrow")
                nc.vector.tensor_copy(out=hrow[:], in_=hpsum[:])
                nc.vector.tensor_copy(
                    out=adst[0][:, t * 4 : (t + 1) * 4], in_=hrow[:, 132:136]
                )
                nc.sync.dma_start(out=shard[0][t * P : (t + 1) * P, :], in_=hrow[:])

            # ---------- per-layer: allgather + edge phase ----------
            for l in range(3):
                if NC_ == 1:  # single-core (simulator) path
                    nc.sync.dma_start(out=table[l][:], in_=shard[l][:])
                else:
                    nc.gpsimd.collective_compute(
                        "AllGather",
                        mybir.AluOpType.bypass,
                        ins=[shard[l][:]],
                        outs=[table_sh[l][:]],
                        replica_groups=rg,
                    )
                    # indirect DMA cannot source a Shared-space tensor
                    # (neuronxcc DataLocalityOpt asserts); stage to local.
                    nc.sync.dma_start(out=table[l][:], in_=table_sh[l][:])
                heads = 4 if l < 2 else 1
                off = 0
                for w in range(NWIN):
                    K = Ks[w]
                    out_raw = pool.tile([P, HID], f32, tag="out_raw")
                    den = pool.tile([P, 4], f32, tag="den")
                    nc.vector.memset(out_raw[:], 0.0)
                    nc.vector.memset(den[:], 0.0)
                    nchunks = (K + KC - 1) // KC
                    for ci in range(nchunks):
                        k0 = ci * KC
                        kn = min(KC, K - k0)
                        g = gpool.tile([P, KC, TW], f32, tag="g")
                        for k in range(kn):
                            nc.gpsimd.indirect_dma_start(
                                out=g[:, k, :],
                                out_offset=None,
                                in_=table[l][:, :],
                                in_offset=bass.IndirectOffsetOnAxis(
                                    ap=idx_sb[:, off + k0 + k : off + k0 + k + 1],
                                    axis=0,
                                ),
                                bounds_check=bound,
                                oob_is_err=False,
                            )
                        # scores -> e [P, heads, kn]
                        e = pool.tile([P, 4, KC], f32, tag="e")
                        for h in range(heads):
                            lr = pool.tile([P, KC], f32, tag="lr")
                            nc.vector.tensor_tensor(
                                out=lr[:, :kn],
                                in0=g[:, :kn, HID + h],
                                in1=adst[l][:, w * 4 + h : w * 4 + h + 1].to_broadcast(
                                    [P, kn]
                                ),
                                op=mybir.AluOpType.add,
                            )
                            lr2 = pool.tile([P, KC], f32, tag="lr2")
                            nc.vector.tensor_scalar_mul(
                                out=lr2[:, :kn], in0=lr[:, :kn], scalar1=NEG_SLOPE
                            )
                            nc.vector.tensor_tensor(
                                out=lr[:, :kn],
                                in0=lr[:, :kn],
                                in1=lr2[:, :kn],
                                op=mybir.AluOpType.max,
                            )
                            nc.scalar.activation(
                                out=e[:, h, :kn], in_=lr[:, :kn], func=AF.Exp
                            )
                        # mask pads
                        nc.vector.tensor_tensor(
                            out=e[:, :heads, :kn],
                            in0=e[:, :heads, :kn],
                            in1=mask_sb[:, off + k0 : off + k0 + kn]
                            .unsqueeze(1)
                            .to_broadcast([P, heads, kn]),
                            op=mybir.AluOpType.mult,
                        )
                        # denom += sum_k e
                        dpart = pool.tile([P, 4], f32, tag="dpart")
                        nc.vector.reduce_sum(
                            out=dpart[:, :heads],
                            in_=e[:, :heads, :kn],
                            axis=mybir.AxisListType.X,
                        )
                        nc.vector.tensor_add(
                            out=den[:, :heads], in0=den[:, :heads], in1=dpart[:, :heads]
                        )
                        # msg = payload * e  (broadcast e over channels, per head)
                        m = mpool.tile([P, KC, HID], f32, tag="m")
                        if heads == 4:
                            for h in range(heads):
                                nc.vector.tensor_tensor(
                                    out=m[:, :kn, h * PER_HEAD : (h + 1) * PER_HEAD],
                                    in0=g[:, :kn, h * PER_HEAD : (h + 1) * PER_HEAD],
                                    in1=e[:, h, :kn]
                                    .unsqueeze(2)
                                    .to_broadcast([P, kn, PER_HEAD]),
                                    op=mybir.AluOpType.mult,
                                )
                        else:
                            nc.vector.tensor_tensor(
                                out=m[:, :kn, :],
                                in0=g[:, :kn, 0:HID],
                                in1=e[:, 0, :kn].unsqueeze(2).to_broadcast([P, kn, HID]),
                                op=mybir.AluOpType.mult,
                            )
                        # out_raw += sum_k m
                        mpart = pool.tile([P, HID], f32, tag="mpart")
                        nc.vector.reduce_sum(
                            out=mpart[:],
                            in_=m[:, :kn, :].transpose([0, 2, 1]),
                            axis=mybir.AxisListType.X,
                        )
                        nc.vector.tensor_add(
                            out=out_raw[:], in0=out_raw[:], in1=mpart[:]
                        )
                    off += K
                    # normalize: out_raw *= 1/(den + eps)
                    rden = pool.tile([P, 4], f32, tag="rden")
                    nc.vector.tensor_scalar_add(out=den[:, :heads], in0=den[:, :heads], scalar1=1e-30)
                    nc.vector.reciprocal(out=rden[:, :heads], in_=den[:, :heads])
                    agg = pool.tile([P, HID], f32, tag="agg")
                    if heads == 4:
                        nc.vector.tensor_tensor(
                            out=agg[:].rearrange("p (h c) -> p h c", h=4),
                            in0=out_raw[:].rearrange("p (h c) -> p h c", h=4),
                            in1=rden[:, :4].unsqueeze(2).to_broadcast([P, 4, PER_HEAD]),
                            op=mybir.AluOpType.mult,
                        )
                    else:
                        nc.vector.tensor_tensor(
                            out=agg[:],
                            in0=out_raw[:],
                            in1=rden[:, 0:1].to_broadcast([P, HID]),
                            op=mybir.AluOpType.mult,
                        )

                    if l < 2:
                        # payload was pre-multiplied (h_l) -> out_l = agg
                        self_out = agg
                        bsb = b1_sb if l == 0 else b2_sb
                    else:
                        # single head: W3 commutes past the aggregation
                        aggT_p = ppt.tile([P, P], f32, tag="aggT_p")
                        nc.tensor.transpose(out=aggT_p[:], in_=agg[:], identity=ident[:])
                        aggT = pool.tile([P, P], f32, tag="aggT")
                        nc.vector.tensor_copy(out=aggT[:], in_=aggT_p[:])
                        opsum = pp.tile([P, TW], f32, tag="hpsum")
                        nc.tensor.matmul(
                            out=opsum[:, :N_CLASSES],
                            lhsT=aggT[:],
                            rhs=w3_sb[:, :N_CLASSES],
                            start=True,
                            stop=True,
                        )
                        self_out = pool.tile([P, HID], f32, tag="self_out")
                        nc.vector.tensor_copy(
                            out=self_out[:, :N_CLASSES], in_=opsum[:, :N_CLASSES]
                        )
                        bsb = b3_sb

                    if l < 2:
                        y = pool.tile([P, HID], f32, tag="y")
                        nc.vector.tensor_add(out=y[:], in0=self_out[:, :HID], in1=bsb[:])
                        neg = pool.tile([P, HID], f32, tag="neg")
                        nc.vector.tensor_scalar_min(out=neg[:], in0=y[:], scalar1=0.0)
                        en = pool.tile([P, HID], f32, tag="en")
                        nc.scalar.activation(out=en[:], in_=neg[:], func=AF.Exp)
                        pos = pool.tile([P, HID], f32, tag="pos")
                        nc.vector.tensor_scalar_max(out=pos[:], in0=y[:], scalar1=0.0)
                        elu = pool.tile([P, HID], f32, tag="elu")
                        nc.vector.tensor_add(out=elu[:], in0=pos[:], in1=en[:])
                        nc.vector.tensor_scalar_add(out=elu[:], in0=elu[:], scalar1=-1.0)
                        # next-layer table row via elu^T
                        eluT_p = ppt.tile([P, P], f32, tag="aggT_p")
                        nc.tensor.transpose(out=eluT_p[:], in_=elu[:], identity=ident[:])
                        eluT = pool.tile([P, P], f32, tag="eluT")
                        nc.vector.tensor_copy(out=eluT[:], in_=eluT_p[:])
                        srow = pool.tile([P, TW], f32, tag="srow")
                        if l == 0:
                            # h2aug = elu1 @ [W2 | W2@as2 | W2@ad2]
                            h2psum = pp.tile([P, TW], f32, tag="hpsum")
                            nc.tensor.matmul(
                                out=h2psum[:],
                                lhsT=eluT[:],
                                rhs=w2aug_sb[:],
                                start=True,
                                stop=True,
                            )
                            nc.vector.tensor_copy(out=srow[:], in_=h2psum[:])
                            nc.vector.tensor_copy(
                                out=adst[1][:, w * 4 : w * 4 + 4], in_=srow[:, 132:136]
                            )
                        else:
                            # layer-3 table: [elu2 | a3src | a3dst]
                            napsum = ppt.tile([P, 8], f32, tag="napsum")
                            nc.tensor.matmul(
                                out=napsum[:],
                                lhsT=eluT[:],
                                rhs=w3a_sb[:],
                                start=True,
                                stop=True,
                            )
                            nc.vector.tensor_copy(out=srow[:, :HID], in_=elu[:])
                            nc.vector.tensor_copy(
                                out=srow[:, HID : HID + 8], in_=napsum[:]
                            )
                            nc.vector.tensor_copy(
                                out=adst[2][:, w * 4 : w * 4 + 1], in_=napsum[:, 1:2]
                            )
                        nc.sync.dma_start(
                            out=shard[l + 1][w * P : (w + 1) * P, :], in_=srow[:]
                        )
                    else:
                        # elu then log_softmax over 9 classes
                        y0 = pool.tile([P, N_CLASSES], f32, tag="y90")
                        nc.vector.tensor_add(
                            out=y0[:], in0=self_out[:, :N_CLASSES], in1=bsb[:]
                        )
                        n9 = pool.tile([P, N_CLASSES], f32, tag="n9")
                        nc.vector.tensor_scalar_min(out=n9[:], in0=y0[:], scalar1=0.0)
                        nc.scalar.activation(out=n9[:], in_=n9[:], func=AF.Exp)
                        p9 = pool.tile([P, N_CLASSES], f32, tag="p9")
                        nc.vector.tensor_scalar_max(out=p9[:], in0=y0[:], scalar1=0.0)
                        y = pool.tile([P, N_CLASSES], f32, tag="y9")
                        nc.vector.tensor_add(out=y[:], in0=p9[:], in1=n9[:])
                        nc.vector.tensor_scalar_add(out=y[:], in0=y[:], scalar1=-1.0)
                        e9 = pool.tile([P, N_CLASSES], f32, tag="e9")
                        nc.scalar.activation(out=e9[:], in_=y[:], func=AF.Exp)
                        s9 = pool.tile([P, 1], f32, tag="s9")
                        nc.vector.reduce_sum(out=s9[:], in_=e9[:], axis=mybir.AxisListType.X)
                        l9 = pool.tile([P, 1], f32, tag="l9")
                        nc.scalar.activation(out=l9[:], in_=s9[:], func=AF.Ln)
                        o9 = pool.tile([P, N_CLASSES], f32, tag="o9")
                        nc.vector.tensor_tensor(
                            out=o9[:],
                            in0=y[:],
                            in1=l9[:].to_broadcast([P, N_CLASSES]),
                            op=mybir.AluOpType.subtract,
                        )
                        nc.sync.dma_start(
                            out=out_ext[w * P : (w + 1) * P, :], in_=o9[:]
                        )
    return nc


# ----------------------------------------------------------------------------
# host wrapper
# ----------------------------------------------------------------------------
def _np(x):
    return np.asarray(x)


def kernel(**inputs):
    from concourse.bass_utils import run_bass_kernel_spmd

    x = _np(inputs["x"]).astype(np.float32)
    edge_index = _np(inputs["edge_index"])
    W1 = _np(inputs["W1"]).astype(np.float32)
    as1 = _np(inputs["as1"]).astype(np.float32)
    ad1 = _np(inputs["ad1"]).astype(np.float32)
    b1 = _np(inputs["b1"]).astype(np.float32)
    W2 = _np(inputs["W2"]).astype(np.float32)
    as2 = _np(inputs["as2"]).astype(np.float32)
    ad2 = _np(inputs["ad2"]).astype(np.float32)
    b2 = _np(inputs["b2"]).astype(np.float32)
    W3 = _np(inputs["W3"]).astype(np.float32)
    as3 = _np(inputs["as3"]).astype(np.float32)
    ad3 = _np(inputs["ad3"]).astype(np.float32)
    b3 = _np(inputs["b3"]).astype(np.float32)

    cores, old_of_new, new_of_old = prep_graph(edge_index)

    # uniform per-window K across cores (SPMD same program)
    Ks = [max(cores[c]["K"][w] for c in range(NC_)) for w in range(NWIN)]
    S = sum(Ks)

    # re-pad each core's idx/mask to uniform window widths
    idx_u = np.full((NC_, P, S), OOB, dtype=np.int32)
    msk_u = np.zeros((NC_, P, S), dtype=np.float32)
    for c in range(NC_):
        off_u = 0
        off_c = 0
        for w in range(NWIN):
            Kc = cores[c]["K"][w]
            Ku = Ks[w]
            if Kc > 0:
                idx_u[c, :, off_u : off_u + Kc] = cores[c]["idx"][:, off_c : off_c + Kc]
                msk_u[c, :, off_u : off_u + Kc] = cores[c]["mask"][:, off_c : off_c + Kc]
            off_u += Ku
            off_c += Kc

    # weights prep
    def blockdiag(a):  # [H, C] -> [H*C, H]
        H, C = a.shape
        out = np.zeros((H * C, H), np.float32)
        for h in range(H):
            out[h * C : (h + 1) * C, h] = a[h]
        return out

    as1b, ad1b = blockdiag(as1), blockdiag(ad1)
    as2b, ad2b = blockdiag(as2), blockdiag(ad2)
    w1aug = np.concatenate([W1, W1 @ as1b, W1 @ ad1b], axis=1).astype(np.float32)
    w2aug = np.concatenate([W2, W2 @ as2b, W2 @ ad2b], axis=1).astype(np.float32)
    w3a = np.zeros((HID, 8), np.float32)
    w3a[:, 0] = (W3 @ as3[0]).astype(np.float32)
    w3a[:, 1] = (W3 @ ad3[0]).astype(np.float32)

    xT = np.zeros((F_IN, NPAD), np.float32)
    real = old_of_new >= 0
    xT[:, real] = x[old_of_new[real]].T

    b1_bc = np.broadcast_to(b1, (P, HID)).copy()
    b2_bc = np.broadcast_to(b2, (P, HID)).copy()
    b3_bc = np.broadcast_to(b3, (P, N_CLASSES)).copy()

    nc = build_nc(Ks, S)
    in_maps = []
    for c in range(NC_):
        in_maps.append(
            {
                "xT": np.ascontiguousarray(xT[:, c * NPER : (c + 1) * NPER]),
                "w1aug": w1aug,
                "w2aug": w2aug,
                "w3": W3,
                "w3a": w3a,
                "bias1": b1_bc,
                "bias2": b2_bc,
                "bias3": b3_bc,
                "idx": idx_u[c],
                "mask": msk_u[c],
            }
        )
    split_excess_waits(nc)
    res = run_bass_kernel_spmd(nc, in_maps, list(range(NC_)))
    out = np.zeros((N, N_CLASSES), np.float32)
    for c in range(NC_):
        rows = old_of_new[c * NPER : (c + 1) * NPER]
        m = rows >= 0
        out[rows[m]] = res.results[c]["out"][m]
    return out

